# revision 38
# baseline (speedup 1.0000x reference)
"""Trainium2 Bass kernel for nn_CrossDConv (sparse deformable attention conv).

Self-contained: host-side sharding/layout prep + Bass/Tile kernel, SPMD on
8 NeuronCores via run_bass_kernel_spmd.  Each core handles one
(batch, row-half) shard of the (4, 64, 64, 64) input.

All device work runs in a width-padded pixel space (66-wide rows, one zero
column each side, plus zero rows above/below the shard) so 3x3-conv taps
and bilinear-gather taps never wrap across rows: zero padding reproduces
the reference's conv zero-padding and zero-padded bilinear sampling
exactly, with no masks.

Math restructuring (exact, host-side):
  * Both depthwise3x3+pointwise1x1 offset branches and the modulation
    branch fuse into ONE composite 3x3 conv producing 104 offset outputs
    (padded to 128 partitions) plus 52 "u" outputs, u = scores - sparsity
    (softmax shift-invariance).  The 9 taps run as 5 matmul passes: 3
    vertical tap-pairs share K=128 via the two row-shifted slab halves of
    x2, and 2 more passes use x3 (middle rows + a 1-column-shifted copy)
    to pair the middle-row taps.  All biases fold into downstream
    activation/vector ops (no ones-row matmuls).
  * Unnormalized softmax weights e = exp(u)*sigmoid(u/tau) computed as
    1/((1+exp(-10u))*exp(-u)) so the scalar engine only ever runs Exp
    (one activation table load for the whole kernel).
  * Bilinear tent weights expanded over monomials {1, relu(t), -relu(-t)}
    computed with fused scalar_tensor_tensor ops; the 3x3 recombination
    and all signs fold into static G matrices.  Monomials are packed in
    pairs on 104 partitions so the 25-tap stencil A_d[p] needs only 5
    PSUM-accumulated G-matmuls; the 26th output row is the softmax
    denominator.
  * 1x1 "pc" conv commutes with the gather: the gather runs on
    y0 = pc_w @ x (computed directly pixel-major); pc bias folds into the
    first MLP bias, mlp_b2 folds into the residual tensor host-side.
  * Gather as banded matmul: normalized pixel-major A scattered into S^T
    (GPSIMD local_scatter, static indices), PE-transposed into q-major S
    chunks, PE matmuls against pixel-major y0.

The pipeline runs as 5 pixel groups (4x512 + 64) so Tile can overlap
phases across groups; all transposes use the PE (DMA-transpose costs
~1.2us of serial Sync-engine dispatch per call on this target).
"""

import numpy as np
import ml_dtypes

import concourse.bass as bass
import concourse.tile as tile
from concourse import mybir, library_config
from concourse.bass_utils import run_bass_kernel_spmd
from concourse.library_overlay import lower_extended_insts

BF16 = mybir.dt.bfloat16
F32 = mybir.dt.float32
I16 = mybir.dt.int16

# ------------------------------------------------------------------ geometry
B, C, H, W = 4, 64, 64, 64
OUTC = 64
N_CORES = 8
TAU = 0.1
NSAMP = 52
WP = W + 2                      # padded row width
ROWS_OUT = H // 2               # 32 output rows per core
LEAD = 63                       # leading zeros so P_OUT0 = 195 (=67+128)
SLAB_ROWS = 40                  # rows r0-2 .. r0+38 (zero-padded outside image)
P_SLAB = 2816                   # 63 + 40*66 + tail zeros, 22 chunks of 128
P_OUT0 = LEAD + 2 * WP          # 195
NP_OUT = ROWS_OUT * WP          # 2112 padded positions carrying outputs
NBLK = (NP_OUT + 127) // 128    # 17 pixel blocks
QSPAN = 512                     # q-window per block: [p0-67, p0+445)
NTAP = 25
NTAPD = 26
NTAPP = 32                      # padded tap stride
SCAT_BLKS = 4                   # max blocks per group
GROUPS = [(0, 4), (4, 4), (8, 4), (12, 4), (16, 1)]   # (block0, nblk)
NSCAT = len(GROUPS)
GCOLS = SCAT_BLKS * 128         # 512 pixels per (full) group

# bf16 weight blobs: small matrices (loaded first, y0 needs pcT) and conv
WS_IDENT = 0                    # [128, 128]
WS_GMAT = 128                   # [52, 9*26]
WS_PCT = 362                    # [64, 64]
WS_W1T = 426
WS_W2T = 490
WS_COLS = 554
WC_COLS = 900                   # wconv [128, 5*180]

# f32 bias blob column layout (per-partition bias vectors)
WF_BOX = 0                      # [52, 1] ox bias
WF_BOY = 1                      # [52, 1] oy bias
WF_NBU = 2                      # [52, 1] -bu
WF_NBU10 = 3                    # [52, 1] -10*bu
WF_B1 = 4                       # [64, 1] mlp bias 1 (incl pc bias)
WF_NBOX = 5                     # [52, 1] -ox bias
WF_NBOY = 6                     # [52, 1] -oy bias
WF_COLS = 8

_CACHE = {}


# =====================================================================
# Device kernel
# =====================================================================

def _emit(nc, tc, d):
    from contextlib import ExitStack

    with ExitStack() as ctx:
        weights = ctx.enter_context(tc.tile_pool(name="weights", bufs=1))
        big = ctx.enter_context(tc.tile_pool(name="big", bufs=1))
        work = ctx.enter_context(tc.tile_pool(name="work", bufs=2))
        mono = ctx.enter_context(tc.tile_pool(name="mono", bufs=2))
        small = ctx.enter_context(tc.tile_pool(name="small", bufs=2))
        schunkp = ctx.enter_context(tc.tile_pool(name="schunk", bufs=3))
        psum = ctx.enter_context(tc.tile_pool(name="psum", bufs=1, space="PSUM"))
        psumA = ctx.enter_context(tc.tile_pool(name="psumA", bufs=1, space="PSUM"))
        psumT = ctx.enter_context(tc.tile_pool(name="psumT", bufs=1, space="PSUM"))

        nc.gpsimd.load_library(library_config.local_scatter)

        # ---------------- loads, ordered so y0 can start early: the small
        # weight blob and the top slab land first; the x2 bottom half is the
        # top shifted 2 rows (132 cols) and x3 pairs the middle-row taps, so
        # both derive from the same DRAM top slab via shifted DMA reads
        wbs = weights.tile([128, WS_COLS], BF16)
        nc.sync.dma_start(out=wbs, in_=d["wbs"][:, :])
        x2 = big.tile([128, P_SLAB], BF16)
        nc.sync.dma_start(out=x2[0:64, :], in_=d["xt"][:, :])
        nc.sync.dma_start(out=x2[64:128, 0 : P_SLAB - 132],
                          in_=d["xt"][:, 132:P_SLAB])
        x3 = big.tile([128, P_SLAB], BF16)
        nc.sync.dma_start(out=x3[0:64, :], in_=d["xt"][:, :])
        nc.sync.dma_start(out=x3[64:128, 0 : P_SLAB - 1],
                          in_=d["xt"][:, 1:P_SLAB])
        wbc = weights.tile([128, WC_COLS], BF16)
        nc.sync.dma_start(out=wbc, in_=d["wbc"][:, :])
        wbf = weights.tile([64, WF_COLS], F32)
        nc.sync.dma_start(out=wbf, in_=d["wbf"][:, :])
        sidx = weights.tile([128, NSCAT, SCAT_BLKS * NTAPP], I16)
        nc.sync.dma_start(out=sidx, in_=d["sidx"][:, :, :])
        xres = big.tile([C, NP_OUT], F32)
        nc.sync.dma_start(out=xres, in_=d["xres"][:, :])

        wconv = wbc.rearrange("p (g m) -> p g m", g=5)
        ident = wbs[:, WS_IDENT:WS_GMAT]
        gmat = wbs[0:NSAMP, WS_GMAT:WS_PCT].rearrange("p (k t) -> p k t", k=9)
        pcT = wbs[0:C, WS_PCT : WS_PCT + 64]
        w1T = wbs[0:OUTC, WS_W1T : WS_W1T + 64]
        w2T = wbs[0:OUTC, WS_W2T : WS_W2T + 64]
        b_ox = wbf[0:NSAMP, WF_BOX : WF_BOX + 1]
        b_oy = wbf[0:NSAMP, WF_BOY : WF_BOY + 1]
        b_nu = wbf[0:NSAMP, WF_NBU : WF_NBU + 1]
        b_nu10 = wbf[0:NSAMP, WF_NBU10 : WF_NBU10 + 1]
        b_1 = wbf[0:OUTC, WF_B1 : WF_B1 + 1]
        b_oxn = wbf[0:NSAMP, WF_NBOX : WF_NBOX + 1]
        b_oyn = wbf[0:NSAMP, WF_NBOY : WF_NBOY + 1]

        # ---------------- y0 pixel-major (chunks 1..20 are used by gather);
        # needs only the top slab + pcT, so it fills the PE while the rest
        # of the inputs stream in
        y0_pm = big.tile([128, 22, OUTC], BF16)

        def emit_y0(qc):
            s = qc * 128
            accy = psum.tile([128, OUTC], F32,
                             tag="ps_mm" if qc % 2 == 0 else "ps_agg")
            nc.tensor.matmul(accy, x2[0:C, s : s + 128], pcT, start=True,
                             stop=True)
            if qc % 2 == 0:
                nc.scalar.activation(y0_pm[:, qc, :], accy,
                                     mybir.ActivationFunctionType.Copy)
            else:
                nc.vector.tensor_copy(y0_pm[:, qc, :], accy)

        for qc in range(1, 21):
            emit_y0(qc)

        # ---------------- main per-group pipeline
        for grp, (blk0, nblk_g) in enumerate(GROUPS):
            gs = blk0 * 128
            ge = min(gs + nblk_g * 128, NP_OUT)
            gn = ge - gs

            # ---- composite conv: 5 matmul passes (3 v-pairs, h-pair, single)
            acc_off = psum.tile([128, GCOLS], F32, tag="ps_coff")
            acc_u = psum.tile([NSAMP, GCOLS], F32, tag="ps_cu")
            for g in range(5):
                if g < 3:
                    tx = g - 1
                    rhs_t, base, kk = x2, P_OUT0 + gs - WP + tx, 128
                elif g == 3:
                    rhs_t, base, kk = x3, P_OUT0 + gs - 1, 128
                else:
                    rhs_t, base, kk = x2, P_OUT0 + gs + 1, 64
                rhs = rhs_t[0:kk, base : base + gn]
                nc.tensor.matmul(acc_off[:, :gn], wconv[:kk, g, 0:128], rhs,
                                 start=(g == 0), stop=False)
                nc.tensor.matmul(acc_u[:, :gn], wconv[:kk, g, 128:180], rhs,
                                 start=(g == 0), stop=(g == 4))

            # offset monomial factors relu(+-ox), relu(+-oy) straight from
            # PSUM on the scalar engine (conv bias folded into the
            # activation; min-branch signs are folded into gmat host-side)
            RELU = mybir.ActivationFunctionType.Relu
            oxp = work.tile([NSAMP, GCOLS], BF16, tag="oxp")
            nc.scalar.activation(oxp[:, :gn], acc_off[0:NSAMP, :gn], RELU,
                                 bias=b_ox)
            oxn = work.tile([NSAMP, GCOLS], BF16, tag="oxn")
            nc.scalar.activation(oxn[:, :gn], acc_off[0:NSAMP, :gn], RELU,
                                 bias=b_oxn, scale=-1.0)
            oyp = work.tile([NSAMP, GCOLS], BF16, tag="oyp")
            nc.scalar.activation(oyp[:, :gn], acc_off[64 : 64 + NSAMP, :gn],
                                 RELU, bias=b_oy)
            oyn = work.tile([NSAMP, GCOLS], BF16, tag="oyn")
            nc.scalar.activation(oyn[:, :gn], acc_off[64 : 64 + NSAMP, :gn],
                                 RELU, bias=b_oyn, scale=-1.0)

            # ---- e = exp(u)*sigmoid(u/tau) = 1/((1+exp(-10u))*exp(-u))
            t3 = work.tile([NSAMP, GCOLS], BF16, tag="t3")
            nc.scalar.activation(t3[:, :gn], acc_u[:, :gn],
                                 mybir.ActivationFunctionType.Exp,
                                 bias=b_nu, scale=-1.0)
            t2 = work.tile([NSAMP, GCOLS], BF16, tag="t2")
            nc.scalar.activation(t2[:, :gn], acc_u[:, :gn],
                                 mybir.ActivationFunctionType.Exp,
                                 bias=b_nu10, scale=-10.0)
            evq = work.tile([NSAMP, GCOLS], F32, tag="evq")
            nc.vector.scalar_tensor_tensor(evq[:, :gn], t2[:, :gn], 1.0,
                                           t3[:, :gn], mybir.AluOpType.add,
                                           mybir.AluOpType.mult)

            # ---- monomials C_ab = e * Ya * Xb (signs folded into gmat);
            # HW requires all SBUF operands of a DVE op to share the same
            # base partition, so each monomial gets its own 52-row tile.
            # evq stays well inside fp32 range (|u| < 1), so the fast
            # Newton-Raphson reciprocal is safe; bf16 cast rides on scalar.
            evf = mono.tile([NSAMP, GCOLS], F32, tag="evf")
            nc.vector.reciprocal_approx_fast(evf[:, :gn], evq[:, :gn])
            ev = mono.tile([NSAMP, GCOLS], BF16, tag="ev")
            nc.scalar.activation(ev[:, :gn], evf[:, :gn],
                                 mybir.ActivationFunctionType.Copy)
            monos = {(0, 0): ev}
            for bb, xf in ((1, oxp), (2, oxn)):
                t = mono.tile([NSAMP, GCOLS], BF16, tag=f"c0{bb}")
                nc.vector.tensor_mul(t[:, :gn], xf[:, :gn], ev[:, :gn])
                monos[(0, bb)] = t
            for bb in range(3):
                t = mono.tile([NSAMP, GCOLS], BF16, tag=f"c1{bb}")
                nc.vector.tensor_mul(t[:, :gn], oyp[:, :gn],
                                     monos[(0, bb)][:, :gn])
                monos[(1, bb)] = t
            for bb in range(3):
                t = mono.tile([NSAMP, GCOLS], BF16, tag=f"c2{bb}")
                nc.vector.tensor_mul(t[:, :gn], oyn[:, :gn],
                                     monos[(0, bb)][:, :gn])
                monos[(2, bb)] = t

            # ---- G-matmuls -> a2 [26, gn] -> a_cm bf16
            a2 = psumA.tile([NTAPD, GCOLS], F32, tag="ps_a2")
            for k in range(9):
                aa, bb = divmod(k, 3)
                nc.tensor.matmul(a2[:, :gn], gmat[:, k, :],
                                 monos[(aa, bb)][:, :gn],
                                 start=(k == 0), stop=(k == 8))
            a_cm = work.tile([NTAPP, GCOLS], BF16, tag="a_cm")
            if gn < GCOLS:
                nc.vector.memset(a_cm, 0.0)
            nc.scalar.activation(a_cm[0:NTAPD, :gn], a2[:, :gn],
                                 mybir.ActivationFunctionType.Copy)

            # ---- pixel-major A via PE transposes
            a_pm_ps = psumT.tile([128, SCAT_BLKS * NTAPP], BF16, tag="ps_apm")
            for bo in range(nblk_g):
                nc.tensor.transpose(a_pm_ps[:, bo * NTAPP : (bo + 1) * NTAPP],
                                    a_cm[:, bo * 128 : (bo + 1) * 128],
                                    ident[0:NTAPP, 0:NTAPP])
            a_pm = work.tile([128, SCAT_BLKS, NTAPP], BF16, tag="a_pm")
            nc.vector.tensor_copy(a_pm[:, 0:nblk_g, :],
                                  a_pm_ps.rearrange("p (b t) -> p b t",
                                                    b=SCAT_BLKS)[:, 0:nblk_g, :])

            # ---- normalize by denominator
            den = small.tile([128, SCAT_BLKS], F32, tag="den")
            nc.vector.tensor_copy(den[:, 0:nblk_g], a_pm[:, 0:nblk_g, 25])
            if gn < nblk_g * 128:
                nc.vector.memset(den[gn - (nblk_g - 1) * 128 :, nblk_g - 1 :
                                     nblk_g], 1.0)
            recip = small.tile([128, SCAT_BLKS], F32, tag="recip")
            nc.vector.reciprocal_approx_fast(recip[:, 0:nblk_g],
                                             den[:, 0:nblk_g])
            for bo in range(nblk_g):
                nc.vector.tensor_scalar_mul(a_pm[:, bo, 0:NTAP],
                                            a_pm[:, bo, 0:NTAP],
                                            recip[:, bo : bo + 1])

            # ---- scatter -> S^T (local_scatter window < 2048 elems, so
            # scatter two blocks at a time; sidx is relative per pair)
            st = work.tile([128, SCAT_BLKS * QSPAN], BF16, tag="st")
            for h in range(0, nblk_g, 2):
                nb = min(2, nblk_g - h)
                nc.gpsimd.local_scatter(
                    st[:, h * QSPAN : (h + nb) * QSPAN],
                    a_pm[:, h : h + nb, :],
                    sidx[:, grp, h * NTAPP : (h + nb) * NTAPP],
                    channels=128, num_elems=nb * QSPAN,
                    num_idxs=nb * NTAPP)

            # ---- gather
            out_cm = work.tile([OUTC, GCOLS], BF16, tag="out_cm")
            for bo in range(nblk_g):
                b = blk0 + bo
                pw = min(128, NP_OUT - b * 128)
                s_ps = psumT.tile([128, 512], BF16, tag="ps_s", bufs=2)
                for qc in range(4):
                    nc.tensor.transpose(
                        s_ps[:, qc * 128 : qc * 128 + pw],
                        st[0:pw, bo * QSPAN + qc * 128 : bo * QSPAN
                           + (qc + 1) * 128],
                        ident[0:pw, 0:pw])
                schunk = schunkp.tile([128, 512], BF16, tag="schunk")
                if bo % 2 == 0:
                    nc.vector.tensor_copy(schunk, s_ps)
                else:
                    nc.scalar.activation(schunk, s_ps,
                                         mybir.ActivationFunctionType.Copy)
                agg = psum.tile([OUTC, 128], F32, tag="ps_agg")
                for qc in range(4):
                    nc.tensor.matmul(agg[:, 0:pw], y0_pm[:, b + 1 + qc, :],
                                     schunk[:, qc * 128 : qc * 128 + pw],
                                     start=(qc == 0), stop=(qc == 3))
                if bo % 2 == 0:
                    nc.scalar.activation(out_cm[:, bo * 128 : bo * 128 + pw],
                                         agg[:, 0:pw],
                                         mybir.ActivationFunctionType.Copy)
                else:
                    nc.vector.tensor_copy(out_cm[:, bo * 128 : bo * 128 + pw],
                                          agg[:, 0:pw])

            # ---- MLP + residual (biases folded into activation / xres)
            acc1 = psum.tile([OUTC, GCOLS], F32, tag="ps_mm")
            nc.tensor.matmul(acc1[:, :gn], w1T, out_cm[:, :gn], start=True,
                             stop=True)
            h1 = work.tile([OUTC, GCOLS], BF16, tag="h1")
            nc.scalar.activation(h1[:, :gn], acc1[:, :gn],
                                 mybir.ActivationFunctionType.Relu,
                                 bias=b_1)
            acc2 = psum.tile([OUTC, GCOLS], F32, tag="ps_mm")
            nc.tensor.matmul(acc2[:, :gn], w2T, h1[:, :gn], start=True,
                             stop=True)
            outt = work.tile([OUTC, GCOLS], F32, tag="outt")
            nc.vector.tensor_add(outt[:, :gn], acc2[:, :gn], xres[:, gs:ge])
            nc.sync.dma_start(out=d["out"][:, gs:ge], in_=outt[:, :gn])


# =====================================================================
# Sync-wait legalizer (walrus CoreV3: max 1 SyncWait per instruction)
# =====================================================================

def _legalize_sync_waits(nc, maxw=1):
    f = nc.m.functions[0]
    inserted = 0
    for bb in list(f.blocks):
        out = []
        changed = False
        for inst in bb.instructions:
            si = inst.sync_info
            if si is not None and si.on_wait and len(si.on_wait) > maxw:
                waits = list(si.on_wait)
                best, order = {}, []
                for w in waits:
                    if w.id not in best:
                        best[w.id] = w
                        order.append(w.id)
                    elif w.wait_value > best[w.id].wait_value:
                        best[w.id] = w
                waits = [best[k] for k in order]
                keep, rest = waits[:maxw], waits[maxw:]
                for w in rest:
                    n = mybir.InstNoOp(name=f"I-lg{nc.next_id()}", ins=[], outs=[])
                    n.engine = inst.engine
                    n.sync_info = mybir.SyncInfo(on_wait=[w], on_update=[])
                    out.append(n)
                    inserted += 1
                si.on_wait = keep
                changed = True
            out.append(inst)
        if changed:
            bb.instructions = out
    return inserted


# =====================================================================
# Host-side preparation
# =====================================================================

def _bf(x):
    return np.ascontiguousarray(np.asarray(x, np.float32).astype(ml_dtypes.bfloat16))


def _f32(x):
    return np.ascontiguousarray(np.asarray(x, np.float32))


def _pad_img(img):
    """(C,H,W) f32 -> (C, H+8, WP) with 4 zero rows top/bottom, 1 col each side."""
    c, h, w = img.shape
    out = np.zeros((c, h + 8, WP), np.float32)
    out[:, 4 : 4 + h, 1 : 1 + w] = img
    return out


def _build_slab(xp, r0):
    """X2 [128, P_SLAB] f32: top = rows [r0-2, r0+38), bottom = top + 2 rows."""
    top = xp[:, r0 + 2 : r0 + 42, :].reshape(C, -1)
    bot = xp[:, r0 + 4 : r0 + 44, :].reshape(C, -1)
    x2 = np.zeros((128, P_SLAB), np.float32)
    x2[0:64, LEAD : LEAD + top.shape[1]] = top
    x2[64:128, LEAD : LEAD + bot.shape[1]] = bot
    return x2


def _tap_deltas():
    return [ty * WP + tx for ty in range(-1, 4) for tx in range(-1, 4)]


def _prep_static(p_n, dwf_w, dwf_b, pwf_w, pwf_b, dwc_w, dwc_b, pwc_w, pwc_b,
                 dwm_w, dwm_b, pwm_w, pwm_b, pc_w, pc_b,
                 mlp_w1, mlp_b1, mlp_w2, mlp_b2):
    p_n = np.asarray(p_n, np.float32)
    px = p_n[0].astype(np.int64)
    py = p_n[1].astype(np.int64)
    assert px.min() >= 0 and px.max() <= 2 and py.min() >= 0 and py.max() <= 2

    # ---- composite conv weights W[tap(3x3), c, m] ----
    P_off = np.concatenate([pwf_w[:, :, 0, 0], pwc_w[:, :, 0, 0]], 0)  # [104, 64]
    nf = pwf_w.shape[0]
    dw_off = np.zeros((104, C, 3, 3), np.float32)
    dw_off[0:nf] = dwf_w[:, 0][None, :, :, :]
    dw_off[nf:104] = dwc_w[:, 0][None, :, :, :]
    db_off = np.zeros((104, C), np.float32)
    db_off[0:nf] = dwf_b[None, :]
    db_off[nf:104] = dwc_b[None, :]

    pwm2 = pwm_w[:, :, 0, 0]
    P_u = pwm2[0:NSAMP] - pwm2[NSAMP : NSAMP + 1]
    b_u0 = pwm_b[0:NSAMP] - pwm_b[NSAMP]

    Wc = np.zeros((9, C, 156), np.float32)
    Bc = np.zeros((156,), np.float32)
    for t in range(9):
        dy, dx = t // 3 - 1, t % 3 - 1
        Wc[t, :, 0:104] = (P_off * dw_off[:, :, dy + 1, dx + 1]).T
        Wc[t, :, 104:156] = (P_u * dwm_w[:, 0, dy + 1, dx + 1][None, :]).T
    Bc[0:104] = np.concatenate([pwf_b, pwc_b]) + (P_off * db_off).sum(1)
    Bc[104:156] = b_u0 + (P_u * dwm_b[None, :]).sum(1)

    # padded M layout: ox at 0:52, oy at 64:116, u separate
    perm = np.zeros((156, 180), np.float32)
    for n in range(NSAMP):
        perm[n, n] = 1.0
        perm[NSAMP + n, 64 + n] = 1.0
        perm[104 + n, 128 + n] = 1.0
    Wcp = np.einsum("tcm,mM->tcM", Wc, perm)
    Bcp = Bc @ perm

    # 5 conv passes: v-pairs (ty=-1 top / ty=+1 bottom), h-pair, single
    wconv = np.zeros((128, 5, 180), np.float32)
    for g in range(3):
        tx = g - 1
        wconv[0:64, g, :] = Wcp[0 * 3 + tx + 1]
        wconv[64:128, g, :] = Wcp[2 * 3 + tx + 1]
    wconv[0:64, 3, :] = Wcp[3]      # (0,-1) on x3 top (middle rows)
    wconv[64:128, 3, :] = Wcp[4]    # (0, 0) on x3 bottom (shifted 1 col)
    wconv[0:64, 4, :] = Wcp[5]      # (0,+1) single, K=64

    # ---- G matrices over monomials ----
    fac = {
        0: {2: -1.0},
        1: {0: 1.0, 1: -1.0, 2: 1.0},
        2: {1: 1.0},
    }
    G = np.zeros((NSAMP, 9, NTAPD), np.float32)
    for n in range(NSAMP):
        for i in range(3):
            for j in range(3):
                ty = py[n] + (i - 1)
                tx = px[n] + (j - 1)
                tap = (ty + 1) * 5 + (tx + 1)
                for a, ca in fac[i].items():
                    for b, cb in fac[j].items():
                        G[n, 3 * a + b, tap] += ca * cb
    # device monomials use relu(-t) instead of min(t,0): flip signs for
    # every index-2 factor
    sgn = np.array([1.0, 1.0, -1.0], np.float32)
    for a in range(3):
        for b in range(3):
            G[:, 3 * a + b, :] *= sgn[a] * sgn[b]
    G[:, 0, 25] = 1.0

    # ---- scatter indices ----
    deltas = _tap_deltas()
    sidx = np.zeros((128, NSCAT, SCAT_BLKS * NTAPP), np.int16)
    for p in range(128):
        negctr = 1
        for sct, (blk0, nblk_g) in enumerate(GROUPS):
            for boff in range(SCAT_BLKS):
                b = blk0 + boff
                for j in range(NTAPP):
                    col = boff * NTAPP + j
                    if boff >= nblk_g or b >= NBLK or j >= NTAP:
                        sidx[p, sct, col] = -negctr
                        negctr += 1
                    else:
                        sidx[p, sct, col] = ((boff % 2) * QSPAN + p
                                             + deltas[j] + 67)
    assert sidx.max() < 2 * QSPAN

    # ---- small weights / bf16 blob ----
    pcT = pc_w[:, :, 0, 0].T
    w1T = mlp_w1.T
    w2T = mlp_w2.T
    b1p = mlp_b1 + mlp_w1 @ pc_b
    b2p = mlp_b2

    wbs = np.zeros((128, WS_COLS), np.float32)
    wbs[:, WS_IDENT:WS_GMAT] = np.eye(128, dtype=np.float32)
    wbs[0:NSAMP, WS_GMAT:WS_PCT] = G.reshape(NSAMP, -1)
    wbs[0:C, WS_PCT : WS_PCT + 64] = pcT
    wbs[0:OUTC, WS_W1T : WS_W1T + 64] = w1T
    wbs[0:OUTC, WS_W2T : WS_W2T + 64] = w2T

    wbf = np.zeros((64, WF_COLS), np.float32)
    wbf[0:NSAMP, WF_BOX] = Bcp[0:NSAMP]
    wbf[0:NSAMP, WF_BOY] = Bcp[64 : 64 + NSAMP]
    wbf[0:NSAMP, WF_NBU] = -Bcp[128:180]
    wbf[0:NSAMP, WF_NBU10] = -10.0 * Bcp[128:180]
    wbf[0:OUTC, WF_B1] = b1p
    wbf[0:NSAMP, WF_NBOX] = -Bcp[0:NSAMP]
    wbf[0:NSAMP, WF_NBOY] = -Bcp[64 : 64 + NSAMP]

    return {
        "wbs": _bf(wbs),
        "wbc": _bf(wconv.reshape(128, -1)),
        "wbf": _f32(wbf),
        "sidx": sidx,
        "b2p": _f32(b2p),
        # logical views for the numpy sim:
        "wconv": wconv,
        "bconv": _f32(Bcp).reshape(180, 1),
        "gmat": G,
        "pcT": pcT,
        "w1T": w1T,
        "w2T": w2T,
        "b1": _f32(b1p).reshape(OUTC, 1),
        "b2": _f32(b2p).reshape(OUTC, 1),
    }


def _build_nc():
    nc = bass.Bass()
    d = {}
    d["xt"] = nc.dram_tensor("xt", [64, P_SLAB], BF16, kind="ExternalInput")
    d["xres"] = nc.dram_tensor("xres", [C, NP_OUT], F32, kind="ExternalInput")
    d["wbs"] = nc.dram_tensor("wbs", [128, WS_COLS], BF16, kind="ExternalInput")
    d["wbc"] = nc.dram_tensor("wbc", [128, WC_COLS], BF16, kind="ExternalInput")
    d["wbf"] = nc.dram_tensor("wbf", [64, WF_COLS], F32, kind="ExternalInput")
    d["sidx"] = nc.dram_tensor("sidx", [128, NSCAT, SCAT_BLKS * NTAPP], I16,
                               kind="ExternalInput")
    d["out"] = nc.dram_tensor("out", [C, NP_OUT], F32, kind="ExternalOutput")

    with tile.TileContext(nc) as tc:
        _emit(nc, tc, d)

    lower_extended_insts(nc)
    _legalize_sync_waits(nc)
    return nc


def _get_nc():
    if "nc" not in _CACHE:
        _CACHE["nc"] = _build_nc()
    return _CACHE["nc"]


def kernel(x, p_n, dwf_w, dwf_b, pwf_w, pwf_b, dwc_w, dwc_b, pwc_w, pwc_b,
           dwm_w, dwm_b, pwm_w, pwm_b, pc_w, pc_b, mlp_w1, mlp_b1, mlp_w2,
           mlp_b2, _bench=None):
    x = np.asarray(x, np.float32)
    stat = _prep_static(
        np.asarray(p_n), np.asarray(dwf_w, np.float32),
        np.asarray(dwf_b, np.float32), np.asarray(pwf_w, np.float32),
        np.asarray(pwf_b, np.float32), np.asarray(dwc_w, np.float32),
        np.asarray(dwc_b, np.float32), np.asarray(pwc_w, np.float32),
        np.asarray(pwc_b, np.float32), np.asarray(dwm_w, np.float32),
        np.asarray(dwm_b, np.float32), np.asarray(pwm_w, np.float32),
        np.asarray(pwm_b, np.float32), np.asarray(pc_w, np.float32),
        np.asarray(pc_b, np.float32), np.asarray(mlp_w1, np.float32),
        np.asarray(mlp_b1, np.float32), np.asarray(mlp_w2, np.float32),
        np.asarray(mlp_b2, np.float32),
    )

    in_maps = []
    shards = []
    for core in range(N_CORES):
        bidx, half = divmod(core, 2)
        r0 = half * ROWS_OUT
        shards.append((bidx, r0))
        xp = _pad_img(x[bidx])
        x2 = _build_slab(xp, r0)
        xres = np.zeros((C, NP_OUT), np.float32)
        xres.reshape(C, ROWS_OUT, WP)[:, :, 1 : 1 + W] = \
            x[bidx, :, r0 : r0 + ROWS_OUT, :]
        xres += stat["b2p"][:, None]
        m = {"wbs": stat["wbs"], "wbc": stat["wbc"], "wbf": stat["wbf"],
             "sidx": stat["sidx"], "xt": _bf(x2[0:64]), "xres": _f32(xres)}
        in_maps.append(m)

    nc = _get_nc()
    kw = dict(_bench) if _bench else {}
    res = run_bass_kernel_spmd(nc, in_maps, list(range(N_CORES)), **kw)

    out = np.zeros((B, OUTC, H, W), np.float32)
    for core, (bidx, r0) in enumerate(shards):
        o = res.results[core]["out"].reshape(OUTC, ROWS_OUT, WP)
        out[bidx, :, r0 : r0 + ROWS_OUT, :] = o[:, :, 1 : 1 + W]
    if _bench is not None:
        _CACHE["last_results"] = res
    return out


# revision 43
# speedup vs baseline: 1.0783x; 1.0783x over previous
"""Trainium2 Bass kernel for nn_CrossDConv (sparse deformable attention conv).

Self-contained: host-side sharding/layout prep + Bass/Tile kernel, SPMD on
8 NeuronCores via run_bass_kernel_spmd.  Each core handles one
(batch, row-half) shard of the (4, 64, 64, 64) input.

All device work runs in a width-padded pixel space (66-wide rows, one zero
column each side, plus zero rows above/below the shard) so 3x3-conv taps
and bilinear-gather taps never wrap across rows: zero padding reproduces
the reference's conv zero-padding and zero-padded bilinear sampling
exactly, with no masks.

Math restructuring (exact, host-side):
  * Both depthwise3x3+pointwise1x1 offset branches and the modulation
    branch fuse into ONE composite 3x3 conv producing 104 offset outputs
    (padded to 128 partitions) plus 52 "u" outputs, u = scores - sparsity
    (softmax shift-invariance).  The 9 taps run as 5 matmul passes: 3
    vertical tap-pairs share K=128 via the two row-shifted slab halves of
    x2, and 2 more passes use x3 (middle rows + a 1-column-shifted copy)
    to pair the middle-row taps.  All biases fold into downstream
    activation/vector ops (no ones-row matmuls).
  * Unnormalized softmax weights e = exp(u)*sigmoid(u/tau) computed as
    1/((1+exp(-10u))*exp(-u)) so the scalar engine only ever runs Exp
    (one activation table load for the whole kernel).
  * Bilinear tent weights expanded over monomials {1, relu(t), -relu(-t)}
    computed with fused scalar_tensor_tensor ops; the 3x3 recombination
    and all signs fold into static G matrices.  Monomials are packed in
    pairs on 104 partitions so the 25-tap stencil A_d[p] needs only 5
    PSUM-accumulated G-matmuls; the 26th output row is the softmax
    denominator.
  * 1x1 "pc" conv commutes with the gather: the gather runs on
    y0 = pc_w @ x (computed directly pixel-major); pc bias folds into the
    first MLP bias, mlp_b2 folds into the residual tensor host-side.
  * Gather as banded matmul: normalized pixel-major A scattered into S^T
    (GPSIMD local_scatter, static indices), PE-transposed into q-major S
    chunks, PE matmuls against pixel-major y0.

The pipeline runs as 5 pixel groups (4x512 + 64) so Tile can overlap
phases across groups; all transposes use the PE (DMA-transpose costs
~1.2us of serial Sync-engine dispatch per call on this target).
"""

import numpy as np
import ml_dtypes

import concourse.bass as bass
import concourse.tile as tile
from concourse import mybir, library_config
from concourse.bass_utils import run_bass_kernel_spmd
from concourse.library_overlay import lower_extended_insts

BF16 = mybir.dt.bfloat16
F32 = mybir.dt.float32
I16 = mybir.dt.int16

# ------------------------------------------------------------------ geometry
B, C, H, W = 4, 64, 64, 64
OUTC = 64
N_CORES = 8
TAU = 0.1
NSAMP = 52
WP = W + 2                      # padded row width
ROWS_OUT = H // 2               # 32 output rows per core
LEAD = 63                       # leading zeros so P_OUT0 = 195 (=67+128)
SLAB_ROWS = 40                  # rows r0-2 .. r0+38 (zero-padded outside image)
P_SLAB = 2816                   # 63 + 40*66 + tail zeros, 22 chunks of 128
P_OUT0 = LEAD + 2 * WP          # 195
NP_OUT = ROWS_OUT * WP          # 2112 padded positions carrying outputs
NBLK = (NP_OUT + 127) // 128    # 17 pixel blocks
QSPAN = 512                     # q-window per block: [p0-67, p0+445)
NTAP = 25
NTAPD = 26
NTAPP = 32                      # padded tap stride
SCAT_BLKS = 4                   # max blocks per group
GROUPS = [(0, 4), (4, 4), (8, 4), (12, 4), (16, 1)]   # (block0, nblk)
NSCAT = len(GROUPS)
GCOLS = SCAT_BLKS * 128         # 512 pixels per (full) group

# bf16 weight blobs: small matrices (loaded first, y0 needs pcT) and conv
WS_IDENT = 0                    # [128, 128]
WS_GMAT = 128                   # [52, 9*26]
WS_PCT = 362                    # [64, 64]
WS_W1T = 426
WS_W2T = 490
WS_COLS = 554
WC_COLS = 900                   # wconv [128, 5*180]

# f32 bias blob column layout (per-partition bias vectors)
WF_BOX = 0                      # [52, 1] ox bias
WF_BOY = 1                      # [52, 1] oy bias
WF_NBU = 2                      # [52, 1] -bu
WF_NBU10 = 3                    # [52, 1] -10*bu
WF_B1 = 4                       # [64, 1] mlp bias 1 (incl pc bias)
WF_NBOX = 5                     # [52, 1] -ox bias
WF_NBOY = 6                     # [52, 1] -oy bias
WF_COLS = 8

_CACHE = {}


# =====================================================================
# Device kernel
# =====================================================================

def _emit(nc, tc, d):
    from contextlib import ExitStack

    with ExitStack() as ctx:
        weights = ctx.enter_context(tc.tile_pool(name="weights", bufs=1))
        big = ctx.enter_context(tc.tile_pool(name="big", bufs=1))
        work = ctx.enter_context(tc.tile_pool(name="work", bufs=2))
        mono = ctx.enter_context(tc.tile_pool(name="mono", bufs=2))
        small = ctx.enter_context(tc.tile_pool(name="small", bufs=2))
        schunkp = ctx.enter_context(tc.tile_pool(name="schunk", bufs=3))
        psum = ctx.enter_context(tc.tile_pool(name="psum", bufs=1, space="PSUM"))
        psumA = ctx.enter_context(tc.tile_pool(name="psumA", bufs=1, space="PSUM"))
        psumT = ctx.enter_context(tc.tile_pool(name="psumT", bufs=1, space="PSUM"))

        nc.gpsimd.load_library(library_config.local_scatter)

        # ---------------- loads, ordered so y0 can start early (it needs
        # only pcT from the small weight blob plus x2); host prebuilds the
        # x2/x3 slab pairs so each lands in one full-width DMA
        wbs = weights.tile([128, WS_COLS], BF16)
        nc.sync.dma_start(out=wbs, in_=d["wbs"][:, :])
        x2 = big.tile([128, P_SLAB], BF16)
        nc.sync.dma_start(out=x2, in_=d["x2"][:, :])
        wbc = weights.tile([128, WC_COLS], BF16)
        nc.sync.dma_start(out=wbc, in_=d["wbc"][:, :])
        x3 = big.tile([128, P_SLAB], BF16)
        nc.sync.dma_start(out=x3, in_=d["x3"][:, :])
        wbf = weights.tile([64, WF_COLS], F32)
        nc.sync.dma_start(out=wbf, in_=d["wbf"][:, :])
        sidx = weights.tile([128, NSCAT, SCAT_BLKS * NTAPP], I16)
        nc.sync.dma_start(out=sidx, in_=d["sidx"][:, :, :])
        xres = big.tile([C, NP_OUT], F32)
        nc.sync.dma_start(out=xres, in_=d["xres"][:, :])

        wconv = wbc.rearrange("p (g m) -> p g m", g=5)
        ident = wbs[:, WS_IDENT:WS_GMAT]
        gmat = wbs[0:NSAMP, WS_GMAT:WS_PCT].rearrange("p (k t) -> p k t", k=9)
        pcT = wbs[0:C, WS_PCT : WS_PCT + 64]
        w1T = wbs[0:OUTC, WS_W1T : WS_W1T + 64]
        w2T = wbs[0:OUTC, WS_W2T : WS_W2T + 64]
        b_ox = wbf[0:NSAMP, WF_BOX : WF_BOX + 1]
        b_oy = wbf[0:NSAMP, WF_BOY : WF_BOY + 1]
        b_nu = wbf[0:NSAMP, WF_NBU : WF_NBU + 1]
        b_nu10 = wbf[0:NSAMP, WF_NBU10 : WF_NBU10 + 1]
        b_1 = wbf[0:OUTC, WF_B1 : WF_B1 + 1]
        b_oxn = wbf[0:NSAMP, WF_NBOX : WF_NBOX + 1]
        b_oyn = wbf[0:NSAMP, WF_NBOY : WF_NBOY + 1]

        # ---------------- y0 pixel-major (chunks 1..20 are used by gather);
        # needs only the top slab + pcT, so it fills the PE while the rest
        # of the inputs stream in
        y0_pm = big.tile([128, 22, OUTC], BF16)

        def emit_y0(qc):
            s = qc * 128
            accy = psum.tile([128, OUTC], F32,
                             tag="ps_mm" if qc % 2 == 0 else "ps_agg")
            nc.tensor.matmul(accy, x2[0:C, s : s + 128], pcT, start=True,
                             stop=True)
            if qc % 2 == 0:
                nc.scalar.activation(y0_pm[:, qc, :], accy,
                                     mybir.ActivationFunctionType.Copy)
            else:
                nc.vector.tensor_copy(y0_pm[:, qc, :], accy)

        for qc in range(1, 9):
            emit_y0(qc)

        # ---------------- main per-group pipeline
        for grp, (blk0, nblk_g) in enumerate(GROUPS):
            gs = blk0 * 128
            ge = min(gs + nblk_g * 128, NP_OUT)
            gn = ge - gs

            # ---- composite conv: 5 matmul passes (3 v-pairs, h-pair, single)
            acc_off = psum.tile([128, GCOLS], F32, tag="ps_coff")
            acc_u = psum.tile([NSAMP, GCOLS], F32, tag="ps_cu")
            for g in range(5):
                if g < 3:
                    tx = g - 1
                    rhs_t, base, kk = x2, P_OUT0 + gs - WP + tx, 128
                elif g == 3:
                    rhs_t, base, kk = x3, P_OUT0 + gs - 1, 128
                else:
                    rhs_t, base, kk = x2, P_OUT0 + gs + 1, 64
                rhs = rhs_t[0:kk, base : base + gn]
                nc.tensor.matmul(acc_off[:, :gn], wconv[:kk, g, 0:128], rhs,
                                 start=(g == 0), stop=False)
                nc.tensor.matmul(acc_u[:, :gn], wconv[:kk, g, 128:180], rhs,
                                 start=(g == 0), stop=(g == 4))

            # deferred y0 chunks fill the PE bubble while the other engines
            # produce group 0's monomials
            if grp == 0:
                for qc in range(9, 21):
                    emit_y0(qc)

            # offset monomial factors relu(+-ox), relu(+-oy) straight from
            # PSUM on the scalar engine (conv bias folded into the
            # activation; min-branch signs are folded into gmat host-side)
            RELU = mybir.ActivationFunctionType.Relu
            oxp = work.tile([NSAMP, GCOLS], BF16, tag="oxp")
            nc.scalar.activation(oxp[:, :gn], acc_off[0:NSAMP, :gn], RELU,
                                 bias=b_ox)
            oxn = work.tile([NSAMP, GCOLS], BF16, tag="oxn")
            nc.scalar.activation(oxn[:, :gn], acc_off[0:NSAMP, :gn], RELU,
                                 bias=b_oxn, scale=-1.0)
            oyp = work.tile([NSAMP, GCOLS], BF16, tag="oyp")
            nc.scalar.activation(oyp[:, :gn], acc_off[64 : 64 + NSAMP, :gn],
                                 RELU, bias=b_oy)
            oyn = work.tile([NSAMP, GCOLS], BF16, tag="oyn")
            nc.scalar.activation(oyn[:, :gn], acc_off[64 : 64 + NSAMP, :gn],
                                 RELU, bias=b_oyn, scale=-1.0)

            # ---- e = exp(u)*sigmoid(u/tau) = 1/((1+exp(-10u))*exp(-u))
            t3 = work.tile([NSAMP, GCOLS], BF16, tag="t3")
            nc.scalar.activation(t3[:, :gn], acc_u[:, :gn],
                                 mybir.ActivationFunctionType.Exp,
                                 bias=b_nu, scale=-1.0)
            t2 = work.tile([NSAMP, GCOLS], BF16, tag="t2")
            nc.scalar.activation(t2[:, :gn], acc_u[:, :gn],
                                 mybir.ActivationFunctionType.Exp,
                                 bias=b_nu10, scale=-10.0)
            evq = work.tile([NSAMP, GCOLS], F32, tag="evq")
            nc.vector.scalar_tensor_tensor(evq[:, :gn], t2[:, :gn], 1.0,
                                           t3[:, :gn], mybir.AluOpType.add,
                                           mybir.AluOpType.mult)

            # ---- monomials C_ab = e * Ya * Xb (signs folded into gmat);
            # HW requires all SBUF operands of a DVE op to share the same
            # base partition, so each monomial gets its own 52-row tile.
            # evq stays well inside fp32 range (|u| < 1), so the fast
            # Newton-Raphson reciprocal is safe; bf16 cast rides on scalar.
            evf = mono.tile([NSAMP, GCOLS], F32, tag="evf")
            nc.vector.reciprocal_approx_fast(evf[:, :gn], evq[:, :gn])
            ev = mono.tile([NSAMP, GCOLS], BF16, tag="ev")
            nc.scalar.activation(ev[:, :gn], evf[:, :gn],
                                 mybir.ActivationFunctionType.Copy)
            monos = {(0, 0): ev}
            for bb, xf in ((1, oxp), (2, oxn)):
                t = mono.tile([NSAMP, GCOLS], BF16, tag=f"c0{bb}")
                nc.vector.tensor_mul(t[:, :gn], xf[:, :gn], ev[:, :gn])
                monos[(0, bb)] = t
            for bb in range(3):
                t = mono.tile([NSAMP, GCOLS], BF16, tag=f"c1{bb}")
                nc.vector.tensor_mul(t[:, :gn], oyp[:, :gn],
                                     monos[(0, bb)][:, :gn])
                monos[(1, bb)] = t
            for bb in range(3):
                t = mono.tile([NSAMP, GCOLS], BF16, tag=f"c2{bb}")
                nc.vector.tensor_mul(t[:, :gn], oyn[:, :gn],
                                     monos[(0, bb)][:, :gn])
                monos[(2, bb)] = t

            # ---- G-matmuls -> a2 [26, gn] -> a_cm bf16
            a2 = psumA.tile([NTAPD, GCOLS], F32, tag="ps_a2")
            for k in range(9):
                aa, bb = divmod(k, 3)
                nc.tensor.matmul(a2[:, :gn], gmat[:, k, :],
                                 monos[(aa, bb)][:, :gn],
                                 start=(k == 0), stop=(k == 8))
            a_cm = work.tile([NTAPP, GCOLS], BF16, tag="a_cm")
            if gn < GCOLS:
                nc.vector.memset(a_cm, 0.0)
            nc.scalar.activation(a_cm[0:NTAPD, :gn], a2[:, :gn],
                                 mybir.ActivationFunctionType.Copy)

            # ---- pixel-major A via PE transposes
            a_pm_ps = psumT.tile([128, SCAT_BLKS * NTAPP], BF16, tag="ps_apm")
            for bo in range(nblk_g):
                nc.tensor.transpose(a_pm_ps[:, bo * NTAPP : (bo + 1) * NTAPP],
                                    a_cm[:, bo * 128 : (bo + 1) * 128],
                                    ident[0:NTAPP, 0:NTAPP])
            a_pm = work.tile([128, SCAT_BLKS, NTAPP], BF16, tag="a_pm")
            nc.vector.tensor_copy(a_pm[:, 0:nblk_g, :],
                                  a_pm_ps.rearrange("p (b t) -> p b t",
                                                    b=SCAT_BLKS)[:, 0:nblk_g, :])

            # ---- normalize by denominator
            den = small.tile([128, SCAT_BLKS], F32, tag="den")
            nc.vector.tensor_copy(den[:, 0:nblk_g], a_pm[:, 0:nblk_g, 25])
            if gn < nblk_g * 128:
                nc.vector.memset(den[gn - (nblk_g - 1) * 128 :, nblk_g - 1 :
                                     nblk_g], 1.0)
            recip = small.tile([128, SCAT_BLKS], F32, tag="recip")
            nc.vector.reciprocal_approx_fast(recip[:, 0:nblk_g],
                                             den[:, 0:nblk_g])
            for bo in range(nblk_g):
                nc.vector.tensor_scalar_mul(a_pm[:, bo, 0:NTAP],
                                            a_pm[:, bo, 0:NTAP],
                                            recip[:, bo : bo + 1])

            # ---- scatter -> S^T (local_scatter window < 2048 elems, so
            # scatter two blocks at a time; sidx is relative per pair)
            st = work.tile([128, SCAT_BLKS * QSPAN], BF16, tag="st")
            for h in range(0, nblk_g, 2):
                nb = min(2, nblk_g - h)
                nc.gpsimd.local_scatter(
                    st[:, h * QSPAN : (h + nb) * QSPAN],
                    a_pm[:, h : h + nb, :],
                    sidx[:, grp, h * NTAPP : (h + nb) * NTAPP],
                    channels=128, num_elems=nb * QSPAN,
                    num_idxs=nb * NTAPP)

            # ---- gather
            out_cm = work.tile([OUTC, GCOLS], BF16, tag="out_cm")
            for bo in range(nblk_g):
                b = blk0 + bo
                pw = min(128, NP_OUT - b * 128)
                s_ps = psumT.tile([128, 512], BF16, tag="ps_s", bufs=2)
                for qc in range(4):
                    nc.tensor.transpose(
                        s_ps[:, qc * 128 : qc * 128 + pw],
                        st[0:pw, bo * QSPAN + qc * 128 : bo * QSPAN
                           + (qc + 1) * 128],
                        ident[0:pw, 0:pw])
                schunk = schunkp.tile([128, 512], BF16, tag="schunk")
                if bo % 2 == 0:
                    nc.vector.tensor_copy(schunk, s_ps)
                else:
                    nc.scalar.activation(schunk, s_ps,
                                         mybir.ActivationFunctionType.Copy)
                agg = psum.tile([OUTC, 128], F32, tag="ps_agg")
                for qc in range(4):
                    nc.tensor.matmul(agg[:, 0:pw], y0_pm[:, b + 1 + qc, :],
                                     schunk[:, qc * 128 : qc * 128 + pw],
                                     start=(qc == 0), stop=(qc == 3))
                if bo % 2 == 0:
                    nc.scalar.activation(out_cm[:, bo * 128 : bo * 128 + pw],
                                         agg[:, 0:pw],
                                         mybir.ActivationFunctionType.Copy)
                else:
                    nc.vector.tensor_copy(out_cm[:, bo * 128 : bo * 128 + pw],
                                          agg[:, 0:pw])

            # ---- MLP + residual (biases folded into activation / xres)
            acc1 = psum.tile([OUTC, GCOLS], F32, tag="ps_mm")
            nc.tensor.matmul(acc1[:, :gn], w1T, out_cm[:, :gn], start=True,
                             stop=True)
            h1 = work.tile([OUTC, GCOLS], BF16, tag="h1")
            nc.scalar.activation(h1[:, :gn], acc1[:, :gn],
                                 mybir.ActivationFunctionType.Relu,
                                 bias=b_1)
            acc2 = psum.tile([OUTC, GCOLS], F32, tag="ps_mm")
            nc.tensor.matmul(acc2[:, :gn], w2T, h1[:, :gn], start=True,
                             stop=True)
            outt = work.tile([OUTC, GCOLS], F32, tag="outt")
            nc.vector.tensor_add(outt[:, :gn], acc2[:, :gn], xres[:, gs:ge])
            nc.sync.dma_start(out=d["out"][:, gs:ge], in_=outt[:, :gn])


# =====================================================================
# Sync-wait legalizer (walrus CoreV3: max 1 SyncWait per instruction)
# =====================================================================

def _legalize_sync_waits(nc, maxw=1):
    f = nc.m.functions[0]
    inserted = 0
    for bb in list(f.blocks):
        out = []
        changed = False
        for inst in bb.instructions:
            si = inst.sync_info
            if si is not None and si.on_wait and len(si.on_wait) > maxw:
                waits = list(si.on_wait)
                best, order = {}, []
                for w in waits:
                    if w.id not in best:
                        best[w.id] = w
                        order.append(w.id)
                    elif w.wait_value > best[w.id].wait_value:
                        best[w.id] = w
                waits = [best[k] for k in order]
                keep, rest = waits[:maxw], waits[maxw:]
                for w in rest:
                    n = mybir.InstNoOp(name=f"I-lg{nc.next_id()}", ins=[], outs=[])
                    n.engine = inst.engine
                    n.sync_info = mybir.SyncInfo(on_wait=[w], on_update=[])
                    out.append(n)
                    inserted += 1
                si.on_wait = keep
                changed = True
            out.append(inst)
        if changed:
            bb.instructions = out
    return inserted


# =====================================================================
# Host-side preparation
# =====================================================================

def _bf(x):
    return np.ascontiguousarray(np.asarray(x, np.float32).astype(ml_dtypes.bfloat16))


def _f32(x):
    return np.ascontiguousarray(np.asarray(x, np.float32))


def _pad_img(img):
    """(C,H,W) f32 -> (C, H+8, WP) with 4 zero rows top/bottom, 1 col each side."""
    c, h, w = img.shape
    out = np.zeros((c, h + 8, WP), np.float32)
    out[:, 4 : 4 + h, 1 : 1 + w] = img
    return out


def _build_slab(xp, r0):
    """X2 [128, P_SLAB] f32: top = rows [r0-2, r0+38), bottom = top + 2 rows."""
    top = xp[:, r0 + 2 : r0 + 42, :].reshape(C, -1)
    bot = xp[:, r0 + 4 : r0 + 44, :].reshape(C, -1)
    x2 = np.zeros((128, P_SLAB), np.float32)
    x2[0:64, LEAD : LEAD + top.shape[1]] = top
    x2[64:128, LEAD : LEAD + bot.shape[1]] = bot
    return x2


def _tap_deltas():
    return [ty * WP + tx for ty in range(-1, 4) for tx in range(-1, 4)]


def _prep_static(p_n, dwf_w, dwf_b, pwf_w, pwf_b, dwc_w, dwc_b, pwc_w, pwc_b,
                 dwm_w, dwm_b, pwm_w, pwm_b, pc_w, pc_b,
                 mlp_w1, mlp_b1, mlp_w2, mlp_b2):
    p_n = np.asarray(p_n, np.float32)
    px = p_n[0].astype(np.int64)
    py = p_n[1].astype(np.int64)
    assert px.min() >= 0 and px.max() <= 2 and py.min() >= 0 and py.max() <= 2

    # ---- composite conv weights W[tap(3x3), c, m] ----
    P_off = np.concatenate([pwf_w[:, :, 0, 0], pwc_w[:, :, 0, 0]], 0)  # [104, 64]
    nf = pwf_w.shape[0]
    dw_off = np.zeros((104, C, 3, 3), np.float32)
    dw_off[0:nf] = dwf_w[:, 0][None, :, :, :]
    dw_off[nf:104] = dwc_w[:, 0][None, :, :, :]
    db_off = np.zeros((104, C), np.float32)
    db_off[0:nf] = dwf_b[None, :]
    db_off[nf:104] = dwc_b[None, :]

    pwm2 = pwm_w[:, :, 0, 0]
    P_u = pwm2[0:NSAMP] - pwm2[NSAMP : NSAMP + 1]
    b_u0 = pwm_b[0:NSAMP] - pwm_b[NSAMP]

    Wc = np.zeros((9, C, 156), np.float32)
    Bc = np.zeros((156,), np.float32)
    for t in range(9):
        dy, dx = t // 3 - 1, t % 3 - 1
        Wc[t, :, 0:104] = (P_off * dw_off[:, :, dy + 1, dx + 1]).T
        Wc[t, :, 104:156] = (P_u * dwm_w[:, 0, dy + 1, dx + 1][None, :]).T
    Bc[0:104] = np.concatenate([pwf_b, pwc_b]) + (P_off * db_off).sum(1)
    Bc[104:156] = b_u0 + (P_u * dwm_b[None, :]).sum(1)

    # padded M layout: ox at 0:52, oy at 64:116, u separate
    perm = np.zeros((156, 180), np.float32)
    for n in range(NSAMP):
        perm[n, n] = 1.0
        perm[NSAMP + n, 64 + n] = 1.0
        perm[104 + n, 128 + n] = 1.0
    Wcp = np.einsum("tcm,mM->tcM", Wc, perm)
    Bcp = Bc @ perm

    # 5 conv passes: v-pairs (ty=-1 top / ty=+1 bottom), h-pair, single
    wconv = np.zeros((128, 5, 180), np.float32)
    for g in range(3):
        tx = g - 1
        wconv[0:64, g, :] = Wcp[0 * 3 + tx + 1]
        wconv[64:128, g, :] = Wcp[2 * 3 + tx + 1]
    wconv[0:64, 3, :] = Wcp[3]      # (0,-1) on x3 top (middle rows)
    wconv[64:128, 3, :] = Wcp[4]    # (0, 0) on x3 bottom (shifted 1 col)
    wconv[0:64, 4, :] = Wcp[5]      # (0,+1) single, K=64

    # ---- G matrices over monomials ----
    fac = {
        0: {2: -1.0},
        1: {0: 1.0, 1: -1.0, 2: 1.0},
        2: {1: 1.0},
    }
    G = np.zeros((NSAMP, 9, NTAPD), np.float32)
    for n in range(NSAMP):
        for i in range(3):
            for j in range(3):
                ty = py[n] + (i - 1)
                tx = px[n] + (j - 1)
                tap = (ty + 1) * 5 + (tx + 1)
                for a, ca in fac[i].items():
                    for b, cb in fac[j].items():
                        G[n, 3 * a + b, tap] += ca * cb
    # device monomials use relu(-t) instead of min(t,0): flip signs for
    # every index-2 factor
    sgn = np.array([1.0, 1.0, -1.0], np.float32)
    for a in range(3):
        for b in range(3):
            G[:, 3 * a + b, :] *= sgn[a] * sgn[b]
    G[:, 0, 25] = 1.0

    # ---- scatter indices ----
    deltas = _tap_deltas()
    sidx = np.zeros((128, NSCAT, SCAT_BLKS * NTAPP), np.int16)
    for p in range(128):
        negctr = 1
        for sct, (blk0, nblk_g) in enumerate(GROUPS):
            for boff in range(SCAT_BLKS):
                b = blk0 + boff
                for j in range(NTAPP):
                    col = boff * NTAPP + j
                    if boff >= nblk_g or b >= NBLK or j >= NTAP:
                        sidx[p, sct, col] = -negctr
                        negctr += 1
                    else:
                        sidx[p, sct, col] = ((boff % 2) * QSPAN + p
                                             + deltas[j] + 67)
    assert sidx.max() < 2 * QSPAN

    # ---- small weights / bf16 blob ----
    pcT = pc_w[:, :, 0, 0].T
    w1T = mlp_w1.T
    w2T = mlp_w2.T
    b1p = mlp_b1 + mlp_w1 @ pc_b
    b2p = mlp_b2

    wbs = np.zeros((128, WS_COLS), np.float32)
    wbs[:, WS_IDENT:WS_GMAT] = np.eye(128, dtype=np.float32)
    wbs[0:NSAMP, WS_GMAT:WS_PCT] = G.reshape(NSAMP, -1)
    wbs[0:C, WS_PCT : WS_PCT + 64] = pcT
    wbs[0:OUTC, WS_W1T : WS_W1T + 64] = w1T
    wbs[0:OUTC, WS_W2T : WS_W2T + 64] = w2T

    wbf = np.zeros((64, WF_COLS), np.float32)
    wbf[0:NSAMP, WF_BOX] = Bcp[0:NSAMP]
    wbf[0:NSAMP, WF_BOY] = Bcp[64 : 64 + NSAMP]
    wbf[0:NSAMP, WF_NBU] = -Bcp[128:180]
    wbf[0:NSAMP, WF_NBU10] = -10.0 * Bcp[128:180]
    wbf[0:OUTC, WF_B1] = b1p
    wbf[0:NSAMP, WF_NBOX] = -Bcp[0:NSAMP]
    wbf[0:NSAMP, WF_NBOY] = -Bcp[64 : 64 + NSAMP]

    return {
        "wbs": _bf(wbs),
        "wbc": _bf(wconv.reshape(128, -1)),
        "wbf": _f32(wbf),
        "sidx": sidx,
        "b2p": _f32(b2p),
        # logical views for the numpy sim:
        "wconv": wconv,
        "bconv": _f32(Bcp).reshape(180, 1),
        "gmat": G,
        "pcT": pcT,
        "w1T": w1T,
        "w2T": w2T,
        "b1": _f32(b1p).reshape(OUTC, 1),
        "b2": _f32(b2p).reshape(OUTC, 1),
    }


def _build_nc():
    nc = bass.Bass()
    d = {}
    d["x2"] = nc.dram_tensor("x2", [128, P_SLAB], BF16, kind="ExternalInput")
    d["x3"] = nc.dram_tensor("x3", [128, P_SLAB], BF16, kind="ExternalInput")
    d["xres"] = nc.dram_tensor("xres", [C, NP_OUT], F32, kind="ExternalInput")
    d["wbs"] = nc.dram_tensor("wbs", [128, WS_COLS], BF16, kind="ExternalInput")
    d["wbc"] = nc.dram_tensor("wbc", [128, WC_COLS], BF16, kind="ExternalInput")
    d["wbf"] = nc.dram_tensor("wbf", [64, WF_COLS], F32, kind="ExternalInput")
    d["sidx"] = nc.dram_tensor("sidx", [128, NSCAT, SCAT_BLKS * NTAPP], I16,
                               kind="ExternalInput")
    d["out"] = nc.dram_tensor("out", [C, NP_OUT], F32, kind="ExternalOutput")

    with tile.TileContext(nc) as tc:
        _emit(nc, tc, d)

    lower_extended_insts(nc)
    _legalize_sync_waits(nc)
    return nc


def _get_nc():
    if "nc" not in _CACHE:
        _CACHE["nc"] = _build_nc()
    return _CACHE["nc"]


def kernel(x, p_n, dwf_w, dwf_b, pwf_w, pwf_b, dwc_w, dwc_b, pwc_w, pwc_b,
           dwm_w, dwm_b, pwm_w, pwm_b, pc_w, pc_b, mlp_w1, mlp_b1, mlp_w2,
           mlp_b2, _bench=None):
    x = np.asarray(x, np.float32)
    stat = _prep_static(
        np.asarray(p_n), np.asarray(dwf_w, np.float32),
        np.asarray(dwf_b, np.float32), np.asarray(pwf_w, np.float32),
        np.asarray(pwf_b, np.float32), np.asarray(dwc_w, np.float32),
        np.asarray(dwc_b, np.float32), np.asarray(pwc_w, np.float32),
        np.asarray(pwc_b, np.float32), np.asarray(dwm_w, np.float32),
        np.asarray(dwm_b, np.float32), np.asarray(pwm_w, np.float32),
        np.asarray(pwm_b, np.float32), np.asarray(pc_w, np.float32),
        np.asarray(pc_b, np.float32), np.asarray(mlp_w1, np.float32),
        np.asarray(mlp_b1, np.float32), np.asarray(mlp_w2, np.float32),
        np.asarray(mlp_b2, np.float32),
    )

    in_maps = []
    shards = []
    for core in range(N_CORES):
        bidx, half = divmod(core, 2)
        r0 = half * ROWS_OUT
        shards.append((bidx, r0))
        xp = _pad_img(x[bidx])
        x2 = _build_slab(xp, r0)
        xres = np.zeros((C, NP_OUT), np.float32)
        xres.reshape(C, ROWS_OUT, WP)[:, :, 1 : 1 + W] = \
            x[bidx, :, r0 : r0 + ROWS_OUT, :]
        xres += stat["b2p"][:, None]
        x3 = np.zeros_like(x2)
        x3[0:64] = x2[0:64]
        x3[64:128, : -1] = x2[0:64, 1:]
        m = {"wbs": stat["wbs"], "wbc": stat["wbc"], "wbf": stat["wbf"],
             "sidx": stat["sidx"], "x2": _bf(x2), "x3": _bf(x3),
             "xres": _f32(xres)}
        in_maps.append(m)

    nc = _get_nc()
    kw = dict(_bench) if _bench else {}
    res = run_bass_kernel_spmd(nc, in_maps, list(range(N_CORES)), **kw)

    out = np.zeros((B, OUTC, H, W), np.float32)
    for core, (bidx, r0) in enumerate(shards):
        o = res.results[core]["out"].reshape(OUTC, ROWS_OUT, WP)
        out[bidx, :, r0 : r0 + ROWS_OUT, :] = o[:, :, 1 : 1 + W]
    if _bench is not None:
        _CACHE["last_results"] = res
    return out


# revision 44
# speedup vs baseline: 1.0791x; 1.0008x over previous
"""Trainium2 Bass kernel for nn_CrossDConv (sparse deformable attention conv).

Self-contained: host-side sharding/layout prep + Bass/Tile kernel, SPMD on
8 NeuronCores via run_bass_kernel_spmd.  Each core handles one
(batch, row-half) shard of the (4, 64, 64, 64) input.

All device work runs in a width-padded pixel space (66-wide rows, one zero
column each side, plus zero rows above/below the shard) so 3x3-conv taps
and bilinear-gather taps never wrap across rows: zero padding reproduces
the reference's conv zero-padding and zero-padded bilinear sampling
exactly, with no masks.

Math restructuring (exact, host-side):
  * Both depthwise3x3+pointwise1x1 offset branches and the modulation
    branch fuse into ONE composite 3x3 conv producing 104 offset outputs
    (padded to 128 partitions) plus 52 "u" outputs, u = scores - sparsity
    (softmax shift-invariance).  The 9 taps run as 5 matmul passes: 3
    vertical tap-pairs share K=128 via the two row-shifted slab halves of
    x2, and 2 more passes use x3 (middle rows + a 1-column-shifted copy)
    to pair the middle-row taps.  All biases fold into downstream
    activation/vector ops (no ones-row matmuls).
  * Unnormalized softmax weights e = exp(u)*sigmoid(u/tau) computed as
    1/((1+exp(-10u))*exp(-u)) so the scalar engine only ever runs Exp
    (one activation table load for the whole kernel).
  * Bilinear tent weights expanded over monomials {1, relu(t), -relu(-t)}
    computed with fused scalar_tensor_tensor ops; the 3x3 recombination
    and all signs fold into static G matrices.  Monomials are packed in
    pairs on 104 partitions so the 25-tap stencil A_d[p] needs only 5
    PSUM-accumulated G-matmuls; the 26th output row is the softmax
    denominator.
  * 1x1 "pc" conv commutes with the gather: the gather runs on
    y0 = pc_w @ x (computed directly pixel-major); pc bias folds into the
    first MLP bias, mlp_b2 folds into the residual tensor host-side.
  * Gather as banded matmul: normalized pixel-major A scattered into S^T
    (GPSIMD local_scatter, static indices), PE-transposed into q-major S
    chunks, PE matmuls against pixel-major y0.

The pipeline runs as 5 pixel groups (4x512 + 64) so Tile can overlap
phases across groups; all transposes use the PE (DMA-transpose costs
~1.2us of serial Sync-engine dispatch per call on this target).
"""

import numpy as np
import ml_dtypes

import concourse.bass as bass
import concourse.tile as tile
from concourse import mybir, library_config
from concourse.bass_utils import run_bass_kernel_spmd
from concourse.library_overlay import lower_extended_insts

BF16 = mybir.dt.bfloat16
F32 = mybir.dt.float32
I16 = mybir.dt.int16

# ------------------------------------------------------------------ geometry
B, C, H, W = 4, 64, 64, 64
OUTC = 64
N_CORES = 8
TAU = 0.1
NSAMP = 52
WP = W + 2                      # padded row width
ROWS_OUT = H // 2               # 32 output rows per core
LEAD = 63                       # leading zeros so P_OUT0 = 195 (=67+128)
SLAB_ROWS = 40                  # rows r0-2 .. r0+38 (zero-padded outside image)
P_SLAB = 2816                   # 63 + 40*66 + tail zeros, 22 chunks of 128
P_OUT0 = LEAD + 2 * WP          # 195
NP_OUT = ROWS_OUT * WP          # 2112 padded positions carrying outputs
NBLK = (NP_OUT + 127) // 128    # 17 pixel blocks
QSPAN = 512                     # q-window per block: [p0-67, p0+445)
NTAP = 25
NTAPD = 26
NTAPP = 32                      # padded tap stride
SCAT_BLKS = 4                   # max blocks per group
GROUPS = [(0, 4), (4, 4), (8, 4), (12, 4), (16, 1)]   # (block0, nblk)
NSCAT = len(GROUPS)
GCOLS = SCAT_BLKS * 128         # 512 pixels per (full) group

# bf16 weight blobs: small matrices (loaded first, y0 needs pcT) and conv
WS_IDENT = 0                    # [128, 128]
WS_GMAT = 128                   # [52, 9*26]
WS_PCT = 362                    # [64, 64]
WS_W1T = 426
WS_W2T = 490
WS_COLS = 554
WC_COLS = 900                   # wconv [128, 5*180]

# f32 bias blob column layout (per-partition bias vectors)
WF_BOX = 0                      # [52, 1] ox bias
WF_BOY = 1                      # [52, 1] oy bias
WF_NBU = 2                      # [52, 1] -bu
WF_NBU10 = 3                    # [52, 1] -10*bu
WF_B1 = 4                       # [64, 1] mlp bias 1 (incl pc bias)
WF_NBOX = 5                     # [52, 1] -ox bias
WF_NBOY = 6                     # [52, 1] -oy bias
WF_COLS = 8

_CACHE = {}


# =====================================================================
# Device kernel
# =====================================================================

def _emit(nc, tc, d):
    from contextlib import ExitStack

    with ExitStack() as ctx:
        weights = ctx.enter_context(tc.tile_pool(name="weights", bufs=1))
        big = ctx.enter_context(tc.tile_pool(name="big", bufs=1))
        work = ctx.enter_context(tc.tile_pool(name="work", bufs=2))
        mono = ctx.enter_context(tc.tile_pool(name="mono", bufs=2))
        small = ctx.enter_context(tc.tile_pool(name="small", bufs=2))
        schunkp = ctx.enter_context(tc.tile_pool(name="schunk", bufs=3))
        psum = ctx.enter_context(tc.tile_pool(name="psum", bufs=1, space="PSUM"))
        psumA = ctx.enter_context(tc.tile_pool(name="psumA", bufs=1, space="PSUM"))
        psumT = ctx.enter_context(tc.tile_pool(name="psumT", bufs=1, space="PSUM"))

        nc.gpsimd.load_library(library_config.local_scatter)

        # ---------------- loads, ordered so y0 can start early (it needs
        # only pcT from the small weight blob plus x2); host prebuilds the
        # x2/x3 slab pairs so each lands in one full-width DMA
        XSPL = 1792                 # y0 chunks 1..8 and conv g0..g2 fit below
        x2 = big.tile([128, P_SLAB], BF16)
        nc.sync.dma_start(out=x2[:, 0:XSPL], in_=d["x2"][:, 0:XSPL])
        wbs = weights.tile([128, WS_COLS], BF16)
        nc.sync.dma_start(out=wbs, in_=d["wbs"][:, :])
        wbc = weights.tile([128, WC_COLS], BF16)
        nc.sync.dma_start(out=wbc, in_=d["wbc"][:, :])
        x3 = big.tile([128, P_SLAB], BF16)
        nc.sync.dma_start(out=x3[:, 0:XSPL], in_=d["x3"][:, 0:XSPL])
        nc.sync.dma_start(out=x2[:, XSPL:P_SLAB], in_=d["x2"][:, XSPL:P_SLAB])
        nc.sync.dma_start(out=x3[:, XSPL:P_SLAB], in_=d["x3"][:, XSPL:P_SLAB])
        wbf = weights.tile([64, WF_COLS], F32)
        nc.sync.dma_start(out=wbf, in_=d["wbf"][:, :])
        sidx = weights.tile([128, NSCAT, SCAT_BLKS * NTAPP], I16)
        nc.sync.dma_start(out=sidx, in_=d["sidx"][:, :, :])
        xres = big.tile([C, NP_OUT], F32)
        nc.sync.dma_start(out=xres, in_=d["xres"][:, :])

        wconv = wbc.rearrange("p (g m) -> p g m", g=5)
        ident = wbs[:, WS_IDENT:WS_GMAT]
        gmat = wbs[0:NSAMP, WS_GMAT:WS_PCT].rearrange("p (k t) -> p k t", k=9)
        pcT = wbs[0:C, WS_PCT : WS_PCT + 64]
        w1T = wbs[0:OUTC, WS_W1T : WS_W1T + 64]
        w2T = wbs[0:OUTC, WS_W2T : WS_W2T + 64]
        b_ox = wbf[0:NSAMP, WF_BOX : WF_BOX + 1]
        b_oy = wbf[0:NSAMP, WF_BOY : WF_BOY + 1]
        b_nu = wbf[0:NSAMP, WF_NBU : WF_NBU + 1]
        b_nu10 = wbf[0:NSAMP, WF_NBU10 : WF_NBU10 + 1]
        b_1 = wbf[0:OUTC, WF_B1 : WF_B1 + 1]
        b_oxn = wbf[0:NSAMP, WF_NBOX : WF_NBOX + 1]
        b_oyn = wbf[0:NSAMP, WF_NBOY : WF_NBOY + 1]

        # ---------------- y0 pixel-major (chunks 1..20 are used by gather);
        # needs only the top slab + pcT, so it fills the PE while the rest
        # of the inputs stream in
        y0_pm = big.tile([128, 22, OUTC], BF16)

        def emit_y0(qc):
            s = qc * 128
            accy = psum.tile([128, OUTC], F32,
                             tag="ps_mm" if qc % 2 == 0 else "ps_agg")
            nc.tensor.matmul(accy, x2[0:C, s : s + 128], pcT, start=True,
                             stop=True)
            if qc % 2 == 0:
                nc.scalar.activation(y0_pm[:, qc, :], accy,
                                     mybir.ActivationFunctionType.Copy)
            else:
                nc.vector.tensor_copy(y0_pm[:, qc, :], accy)

        for qc in range(1, 9):
            emit_y0(qc)

        # ---------------- main per-group pipeline
        for grp, (blk0, nblk_g) in enumerate(GROUPS):
            gs = blk0 * 128
            ge = min(gs + nblk_g * 128, NP_OUT)
            gn = ge - gs

            # ---- composite conv: 5 matmul passes (3 v-pairs, h-pair, single)
            acc_off = psum.tile([128, GCOLS], F32, tag="ps_coff")
            acc_u = psum.tile([NSAMP, GCOLS], F32, tag="ps_cu")
            for g in range(5):
                if g < 3:
                    tx = g - 1
                    rhs_t, base, kk = x2, P_OUT0 + gs - WP + tx, 128
                elif g == 3:
                    rhs_t, base, kk = x3, P_OUT0 + gs - 1, 128
                else:
                    rhs_t, base, kk = x2, P_OUT0 + gs + 1, 64
                rhs = rhs_t[0:kk, base : base + gn]
                nc.tensor.matmul(acc_off[:, :gn], wconv[:kk, g, 0:128], rhs,
                                 start=(g == 0), stop=False)
                nc.tensor.matmul(acc_u[:, :gn], wconv[:kk, g, 128:180], rhs,
                                 start=(g == 0), stop=(g == 4))

            # deferred y0 chunks fill the PE bubble while the other engines
            # produce group 0's monomials
            if grp == 0:
                for qc in range(9, 21):
                    emit_y0(qc)

            # offset monomial factors relu(+-ox), relu(+-oy) straight from
            # PSUM on the scalar engine (conv bias folded into the
            # activation; min-branch signs are folded into gmat host-side)
            RELU = mybir.ActivationFunctionType.Relu
            oxp = work.tile([NSAMP, GCOLS], BF16, tag="oxp")
            nc.scalar.activation(oxp[:, :gn], acc_off[0:NSAMP, :gn], RELU,
                                 bias=b_ox)
            oxn = work.tile([NSAMP, GCOLS], BF16, tag="oxn")
            nc.scalar.activation(oxn[:, :gn], acc_off[0:NSAMP, :gn], RELU,
                                 bias=b_oxn, scale=-1.0)
            oyp = work.tile([NSAMP, GCOLS], BF16, tag="oyp")
            nc.scalar.activation(oyp[:, :gn], acc_off[64 : 64 + NSAMP, :gn],
                                 RELU, bias=b_oy)
            oyn = work.tile([NSAMP, GCOLS], BF16, tag="oyn")
            nc.scalar.activation(oyn[:, :gn], acc_off[64 : 64 + NSAMP, :gn],
                                 RELU, bias=b_oyn, scale=-1.0)

            # ---- e = exp(u)*sigmoid(u/tau) = 1/((1+exp(-10u))*exp(-u))
            t3 = work.tile([NSAMP, GCOLS], BF16, tag="t3")
            nc.scalar.activation(t3[:, :gn], acc_u[:, :gn],
                                 mybir.ActivationFunctionType.Exp,
                                 bias=b_nu, scale=-1.0)
            t2 = work.tile([NSAMP, GCOLS], BF16, tag="t2")
            nc.scalar.activation(t2[:, :gn], acc_u[:, :gn],
                                 mybir.ActivationFunctionType.Exp,
                                 bias=b_nu10, scale=-10.0)
            evq = work.tile([NSAMP, GCOLS], F32, tag="evq")
            nc.vector.scalar_tensor_tensor(evq[:, :gn], t2[:, :gn], 1.0,
                                           t3[:, :gn], mybir.AluOpType.add,
                                           mybir.AluOpType.mult)

            # ---- monomials C_ab = e * Ya * Xb (signs folded into gmat);
            # HW requires all SBUF operands of a DVE op to share the same
            # base partition, so each monomial gets its own 52-row tile.
            # evq stays well inside fp32 range (|u| < 1), so the fast
            # Newton-Raphson reciprocal is safe; bf16 cast rides on scalar.
            evf = mono.tile([NSAMP, GCOLS], F32, tag="evf")
            nc.vector.reciprocal_approx_fast(evf[:, :gn], evq[:, :gn])
            ev = mono.tile([NSAMP, GCOLS], BF16, tag="ev")
            nc.scalar.activation(ev[:, :gn], evf[:, :gn],
                                 mybir.ActivationFunctionType.Copy)
            monos = {(0, 0): ev}
            for bb, xf in ((1, oxp), (2, oxn)):
                t = mono.tile([NSAMP, GCOLS], BF16, tag=f"c0{bb}")
                nc.vector.tensor_mul(t[:, :gn], xf[:, :gn], ev[:, :gn])
                monos[(0, bb)] = t
            for bb in range(3):
                t = mono.tile([NSAMP, GCOLS], BF16, tag=f"c1{bb}")
                nc.vector.tensor_mul(t[:, :gn], oyp[:, :gn],
                                     monos[(0, bb)][:, :gn])
                monos[(1, bb)] = t
            for bb in range(3):
                t = mono.tile([NSAMP, GCOLS], BF16, tag=f"c2{bb}")
                nc.vector.tensor_mul(t[:, :gn], oyn[:, :gn],
                                     monos[(0, bb)][:, :gn])
                monos[(2, bb)] = t

            # ---- G-matmuls -> a2 [26, gn] -> a_cm bf16
            a2 = psumA.tile([NTAPD, GCOLS], F32, tag="ps_a2")
            for k in range(9):
                aa, bb = divmod(k, 3)
                nc.tensor.matmul(a2[:, :gn], gmat[:, k, :],
                                 monos[(aa, bb)][:, :gn],
                                 start=(k == 0), stop=(k == 8))
            a_cm = work.tile([NTAPP, GCOLS], BF16, tag="a_cm")
            if gn < GCOLS:
                nc.vector.memset(a_cm, 0.0)
            nc.scalar.activation(a_cm[0:NTAPD, :gn], a2[:, :gn],
                                 mybir.ActivationFunctionType.Copy)

            # ---- pixel-major A via PE transposes
            a_pm_ps = psumT.tile([128, SCAT_BLKS * NTAPP], BF16, tag="ps_apm")
            for bo in range(nblk_g):
                nc.tensor.transpose(a_pm_ps[:, bo * NTAPP : (bo + 1) * NTAPP],
                                    a_cm[:, bo * 128 : (bo + 1) * 128],
                                    ident[0:NTAPP, 0:NTAPP])
            a_pm = work.tile([128, SCAT_BLKS, NTAPP], BF16, tag="a_pm")
            nc.vector.tensor_copy(a_pm[:, 0:nblk_g, :],
                                  a_pm_ps.rearrange("p (b t) -> p b t",
                                                    b=SCAT_BLKS)[:, 0:nblk_g, :])

            # ---- normalize by denominator
            den = small.tile([128, SCAT_BLKS], F32, tag="den")
            nc.vector.tensor_copy(den[:, 0:nblk_g], a_pm[:, 0:nblk_g, 25])
            if gn < nblk_g * 128:
                nc.vector.memset(den[gn - (nblk_g - 1) * 128 :, nblk_g - 1 :
                                     nblk_g], 1.0)
            recip = small.tile([128, SCAT_BLKS], F32, tag="recip")
            nc.vector.reciprocal_approx_fast(recip[:, 0:nblk_g],
                                             den[:, 0:nblk_g])
            for bo in range(nblk_g):
                nc.vector.tensor_scalar_mul(a_pm[:, bo, 0:NTAP],
                                            a_pm[:, bo, 0:NTAP],
                                            recip[:, bo : bo + 1])

            # ---- scatter -> S^T (local_scatter window < 2048 elems, so
            # scatter two blocks at a time; sidx is relative per pair)
            st = work.tile([128, SCAT_BLKS * QSPAN], BF16, tag="st")
            for h in range(0, nblk_g, 2):
                nb = min(2, nblk_g - h)
                nc.gpsimd.local_scatter(
                    st[:, h * QSPAN : (h + nb) * QSPAN],
                    a_pm[:, h : h + nb, :],
                    sidx[:, grp, h * NTAPP : (h + nb) * NTAPP],
                    channels=128, num_elems=nb * QSPAN,
                    num_idxs=nb * NTAPP)

            # ---- gather
            out_cm = work.tile([OUTC, GCOLS], BF16, tag="out_cm")
            for bo in range(nblk_g):
                b = blk0 + bo
                pw = min(128, NP_OUT - b * 128)
                s_ps = psumT.tile([128, 512], BF16, tag="ps_s", bufs=2)
                for qc in range(4):
                    nc.tensor.transpose(
                        s_ps[:, qc * 128 : qc * 128 + pw],
                        st[0:pw, bo * QSPAN + qc * 128 : bo * QSPAN
                           + (qc + 1) * 128],
                        ident[0:pw, 0:pw])
                schunk = schunkp.tile([128, 512], BF16, tag="schunk")
                if bo % 2 == 0:
                    nc.vector.tensor_copy(schunk, s_ps)
                else:
                    nc.scalar.activation(schunk, s_ps,
                                         mybir.ActivationFunctionType.Copy)
                agg = psum.tile([OUTC, 128], F32, tag="ps_agg")
                for qc in range(4):
                    nc.tensor.matmul(agg[:, 0:pw], y0_pm[:, b + 1 + qc, :],
                                     schunk[:, qc * 128 : qc * 128 + pw],
                                     start=(qc == 0), stop=(qc == 3))
                if bo % 2 == 0:
                    nc.scalar.activation(out_cm[:, bo * 128 : bo * 128 + pw],
                                         agg[:, 0:pw],
                                         mybir.ActivationFunctionType.Copy)
                else:
                    nc.vector.tensor_copy(out_cm[:, bo * 128 : bo * 128 + pw],
                                          agg[:, 0:pw])

            # ---- MLP + residual (biases folded into activation / xres)
            acc1 = psum.tile([OUTC, GCOLS], F32, tag="ps_mm")
            nc.tensor.matmul(acc1[:, :gn], w1T, out_cm[:, :gn], start=True,
                             stop=True)
            h1 = work.tile([OUTC, GCOLS], BF16, tag="h1")
            nc.scalar.activation(h1[:, :gn], acc1[:, :gn],
                                 mybir.ActivationFunctionType.Relu,
                                 bias=b_1)
            acc2 = psum.tile([OUTC, GCOLS], F32, tag="ps_mm")
            nc.tensor.matmul(acc2[:, :gn], w2T, h1[:, :gn], start=True,
                             stop=True)
            outt = work.tile([OUTC, GCOLS], F32, tag="outt")
            nc.vector.tensor_add(outt[:, :gn], acc2[:, :gn], xres[:, gs:ge])
            nc.sync.dma_start(out=d["out"][:, gs:ge], in_=outt[:, :gn])


# =====================================================================
# Sync-wait legalizer (walrus CoreV3: max 1 SyncWait per instruction)
# =====================================================================

def _legalize_sync_waits(nc, maxw=1):
    f = nc.m.functions[0]
    inserted = 0
    for bb in list(f.blocks):
        out = []
        changed = False
        for inst in bb.instructions:
            si = inst.sync_info
            if si is not None and si.on_wait and len(si.on_wait) > maxw:
                waits = list(si.on_wait)
                best, order = {}, []
                for w in waits:
                    if w.id not in best:
                        best[w.id] = w
                        order.append(w.id)
                    elif w.wait_value > best[w.id].wait_value:
                        best[w.id] = w
                waits = [best[k] for k in order]
                keep, rest = waits[:maxw], waits[maxw:]
                for w in rest:
                    n = mybir.InstNoOp(name=f"I-lg{nc.next_id()}", ins=[], outs=[])
                    n.engine = inst.engine
                    n.sync_info = mybir.SyncInfo(on_wait=[w], on_update=[])
                    out.append(n)
                    inserted += 1
                si.on_wait = keep
                changed = True
            out.append(inst)
        if changed:
            bb.instructions = out
    return inserted


# =====================================================================
# Host-side preparation
# =====================================================================

def _bf(x):
    return np.ascontiguousarray(np.asarray(x, np.float32).astype(ml_dtypes.bfloat16))


def _f32(x):
    return np.ascontiguousarray(np.asarray(x, np.float32))


def _pad_img(img):
    """(C,H,W) f32 -> (C, H+8, WP) with 4 zero rows top/bottom, 1 col each side."""
    c, h, w = img.shape
    out = np.zeros((c, h + 8, WP), np.float32)
    out[:, 4 : 4 + h, 1 : 1 + w] = img
    return out


def _build_slab(xp, r0):
    """X2 [128, P_SLAB] f32: top = rows [r0-2, r0+38), bottom = top + 2 rows."""
    top = xp[:, r0 + 2 : r0 + 42, :].reshape(C, -1)
    bot = xp[:, r0 + 4 : r0 + 44, :].reshape(C, -1)
    x2 = np.zeros((128, P_SLAB), np.float32)
    x2[0:64, LEAD : LEAD + top.shape[1]] = top
    x2[64:128, LEAD : LEAD + bot.shape[1]] = bot
    return x2


def _tap_deltas():
    return [ty * WP + tx for ty in range(-1, 4) for tx in range(-1, 4)]


def _prep_static(p_n, dwf_w, dwf_b, pwf_w, pwf_b, dwc_w, dwc_b, pwc_w, pwc_b,
                 dwm_w, dwm_b, pwm_w, pwm_b, pc_w, pc_b,
                 mlp_w1, mlp_b1, mlp_w2, mlp_b2):
    p_n = np.asarray(p_n, np.float32)
    px = p_n[0].astype(np.int64)
    py = p_n[1].astype(np.int64)
    assert px.min() >= 0 and px.max() <= 2 and py.min() >= 0 and py.max() <= 2

    # ---- composite conv weights W[tap(3x3), c, m] ----
    P_off = np.concatenate([pwf_w[:, :, 0, 0], pwc_w[:, :, 0, 0]], 0)  # [104, 64]
    nf = pwf_w.shape[0]
    dw_off = np.zeros((104, C, 3, 3), np.float32)
    dw_off[0:nf] = dwf_w[:, 0][None, :, :, :]
    dw_off[nf:104] = dwc_w[:, 0][None, :, :, :]
    db_off = np.zeros((104, C), np.float32)
    db_off[0:nf] = dwf_b[None, :]
    db_off[nf:104] = dwc_b[None, :]

    pwm2 = pwm_w[:, :, 0, 0]
    P_u = pwm2[0:NSAMP] - pwm2[NSAMP : NSAMP + 1]
    b_u0 = pwm_b[0:NSAMP] - pwm_b[NSAMP]

    Wc = np.zeros((9, C, 156), np.float32)
    Bc = np.zeros((156,), np.float32)
    for t in range(9):
        dy, dx = t // 3 - 1, t % 3 - 1
        Wc[t, :, 0:104] = (P_off * dw_off[:, :, dy + 1, dx + 1]).T
        Wc[t, :, 104:156] = (P_u * dwm_w[:, 0, dy + 1, dx + 1][None, :]).T
    Bc[0:104] = np.concatenate([pwf_b, pwc_b]) + (P_off * db_off).sum(1)
    Bc[104:156] = b_u0 + (P_u * dwm_b[None, :]).sum(1)

    # padded M layout: ox at 0:52, oy at 64:116, u separate
    perm = np.zeros((156, 180), np.float32)
    for n in range(NSAMP):
        perm[n, n] = 1.0
        perm[NSAMP + n, 64 + n] = 1.0
        perm[104 + n, 128 + n] = 1.0
    Wcp = np.einsum("tcm,mM->tcM", Wc, perm)
    Bcp = Bc @ perm

    # 5 conv passes: v-pairs (ty=-1 top / ty=+1 bottom), h-pair, single
    wconv = np.zeros((128, 5, 180), np.float32)
    for g in range(3):
        tx = g - 1
        wconv[0:64, g, :] = Wcp[0 * 3 + tx + 1]
        wconv[64:128, g, :] = Wcp[2 * 3 + tx + 1]
    wconv[0:64, 3, :] = Wcp[3]      # (0,-1) on x3 top (middle rows)
    wconv[64:128, 3, :] = Wcp[4]    # (0, 0) on x3 bottom (shifted 1 col)
    wconv[0:64, 4, :] = Wcp[5]      # (0,+1) single, K=64

    # ---- G matrices over monomials ----
    fac = {
        0: {2: -1.0},
        1: {0: 1.0, 1: -1.0, 2: 1.0},
        2: {1: 1.0},
    }
    G = np.zeros((NSAMP, 9, NTAPD), np.float32)
    for n in range(NSAMP):
        for i in range(3):
            for j in range(3):
                ty = py[n] + (i - 1)
                tx = px[n] + (j - 1)
                tap = (ty + 1) * 5 + (tx + 1)
                for a, ca in fac[i].items():
                    for b, cb in fac[j].items():
                        G[n, 3 * a + b, tap] += ca * cb
    # device monomials use relu(-t) instead of min(t,0): flip signs for
    # every index-2 factor
    sgn = np.array([1.0, 1.0, -1.0], np.float32)
    for a in range(3):
        for b in range(3):
            G[:, 3 * a + b, :] *= sgn[a] * sgn[b]
    G[:, 0, 25] = 1.0

    # ---- scatter indices ----
    deltas = _tap_deltas()
    sidx = np.zeros((128, NSCAT, SCAT_BLKS * NTAPP), np.int16)
    for p in range(128):
        negctr = 1
        for sct, (blk0, nblk_g) in enumerate(GROUPS):
            for boff in range(SCAT_BLKS):
                b = blk0 + boff
                for j in range(NTAPP):
                    col = boff * NTAPP + j
                    if boff >= nblk_g or b >= NBLK or j >= NTAP:
                        sidx[p, sct, col] = -negctr
                        negctr += 1
                    else:
                        sidx[p, sct, col] = ((boff % 2) * QSPAN + p
                                             + deltas[j] + 67)
    assert sidx.max() < 2 * QSPAN

    # ---- small weights / bf16 blob ----
    pcT = pc_w[:, :, 0, 0].T
    w1T = mlp_w1.T
    w2T = mlp_w2.T
    b1p = mlp_b1 + mlp_w1 @ pc_b
    b2p = mlp_b2

    wbs = np.zeros((128, WS_COLS), np.float32)
    wbs[:, WS_IDENT:WS_GMAT] = np.eye(128, dtype=np.float32)
    wbs[0:NSAMP, WS_GMAT:WS_PCT] = G.reshape(NSAMP, -1)
    wbs[0:C, WS_PCT : WS_PCT + 64] = pcT
    wbs[0:OUTC, WS_W1T : WS_W1T + 64] = w1T
    wbs[0:OUTC, WS_W2T : WS_W2T + 64] = w2T

    wbf = np.zeros((64, WF_COLS), np.float32)
    wbf[0:NSAMP, WF_BOX] = Bcp[0:NSAMP]
    wbf[0:NSAMP, WF_BOY] = Bcp[64 : 64 + NSAMP]
    wbf[0:NSAMP, WF_NBU] = -Bcp[128:180]
    wbf[0:NSAMP, WF_NBU10] = -10.0 * Bcp[128:180]
    wbf[0:OUTC, WF_B1] = b1p
    wbf[0:NSAMP, WF_NBOX] = -Bcp[0:NSAMP]
    wbf[0:NSAMP, WF_NBOY] = -Bcp[64 : 64 + NSAMP]

    return {
        "wbs": _bf(wbs),
        "wbc": _bf(wconv.reshape(128, -1)),
        "wbf": _f32(wbf),
        "sidx": sidx,
        "b2p": _f32(b2p),
        # logical views for the numpy sim:
        "wconv": wconv,
        "bconv": _f32(Bcp).reshape(180, 1),
        "gmat": G,
        "pcT": pcT,
        "w1T": w1T,
        "w2T": w2T,
        "b1": _f32(b1p).reshape(OUTC, 1),
        "b2": _f32(b2p).reshape(OUTC, 1),
    }


def _build_nc():
    nc = bass.Bass()
    d = {}
    d["x2"] = nc.dram_tensor("x2", [128, P_SLAB], BF16, kind="ExternalInput")
    d["x3"] = nc.dram_tensor("x3", [128, P_SLAB], BF16, kind="ExternalInput")
    d["xres"] = nc.dram_tensor("xres", [C, NP_OUT], F32, kind="ExternalInput")
    d["wbs"] = nc.dram_tensor("wbs", [128, WS_COLS], BF16, kind="ExternalInput")
    d["wbc"] = nc.dram_tensor("wbc", [128, WC_COLS], BF16, kind="ExternalInput")
    d["wbf"] = nc.dram_tensor("wbf", [64, WF_COLS], F32, kind="ExternalInput")
    d["sidx"] = nc.dram_tensor("sidx", [128, NSCAT, SCAT_BLKS * NTAPP], I16,
                               kind="ExternalInput")
    d["out"] = nc.dram_tensor("out", [C, NP_OUT], F32, kind="ExternalOutput")

    with tile.TileContext(nc) as tc:
        _emit(nc, tc, d)

    lower_extended_insts(nc)
    _legalize_sync_waits(nc)
    return nc


def _get_nc():
    if "nc" not in _CACHE:
        _CACHE["nc"] = _build_nc()
    return _CACHE["nc"]


def kernel(x, p_n, dwf_w, dwf_b, pwf_w, pwf_b, dwc_w, dwc_b, pwc_w, pwc_b,
           dwm_w, dwm_b, pwm_w, pwm_b, pc_w, pc_b, mlp_w1, mlp_b1, mlp_w2,
           mlp_b2, _bench=None):
    x = np.asarray(x, np.float32)
    stat = _prep_static(
        np.asarray(p_n), np.asarray(dwf_w, np.float32),
        np.asarray(dwf_b, np.float32), np.asarray(pwf_w, np.float32),
        np.asarray(pwf_b, np.float32), np.asarray(dwc_w, np.float32),
        np.asarray(dwc_b, np.float32), np.asarray(pwc_w, np.float32),
        np.asarray(pwc_b, np.float32), np.asarray(dwm_w, np.float32),
        np.asarray(dwm_b, np.float32), np.asarray(pwm_w, np.float32),
        np.asarray(pwm_b, np.float32), np.asarray(pc_w, np.float32),
        np.asarray(pc_b, np.float32), np.asarray(mlp_w1, np.float32),
        np.asarray(mlp_b1, np.float32), np.asarray(mlp_w2, np.float32),
        np.asarray(mlp_b2, np.float32),
    )

    in_maps = []
    shards = []
    for core in range(N_CORES):
        bidx, half = divmod(core, 2)
        r0 = half * ROWS_OUT
        shards.append((bidx, r0))
        xp = _pad_img(x[bidx])
        x2 = _build_slab(xp, r0)
        xres = np.zeros((C, NP_OUT), np.float32)
        xres.reshape(C, ROWS_OUT, WP)[:, :, 1 : 1 + W] = \
            x[bidx, :, r0 : r0 + ROWS_OUT, :]
        xres += stat["b2p"][:, None]
        x3 = np.zeros_like(x2)
        x3[0:64] = x2[0:64]
        x3[64:128, : -1] = x2[0:64, 1:]
        m = {"wbs": stat["wbs"], "wbc": stat["wbc"], "wbf": stat["wbf"],
             "sidx": stat["sidx"], "x2": _bf(x2), "x3": _bf(x3),
             "xres": _f32(xres)}
        in_maps.append(m)

    nc = _get_nc()
    kw = dict(_bench) if _bench else {}
    res = run_bass_kernel_spmd(nc, in_maps, list(range(N_CORES)), **kw)

    out = np.zeros((B, OUTC, H, W), np.float32)
    for core, (bidx, r0) in enumerate(shards):
        o = res.results[core]["out"].reshape(OUTC, ROWS_OUT, WP)
        out[bidx, :, r0 : r0 + ROWS_OUT, :] = o[:, :, 1 : 1 + W]
    if _bench is not None:
        _CACHE["last_results"] = res
    return out
